# revision 27
# baseline (speedup 1.0000x reference)
"""Trainium2 Bass kernel for a 2-layer dense GCN block:

    z = x.reshape(B, N, F)                     # B=4, N=8192, F=64
    for i in range(2):
        z = relu((A @ z) @ W_i)                # A: [N, N] dense
    return z

Strategy (8 NeuronCores, SPMD):
  * Shard the output rows (m) of A @ Z across cores: core j owns rows
    [1024*j, 1024*(j+1)).  The host hands core j the matching
    column-slice of A^T (contraction dim n on SBUF partitions), cast to
    bf16 and pre-swizzled into exact SBUF tile order so every chunk DMA
    is one flat contiguous copy.  The 16 MB shard stays resident in
    SBUF for BOTH layers -- A is read from HBM exactly once.
  * Z is a [n, c] matrix with c = b*F + f (256 columns).  Layer matmuls
    compute H^T[c, m] = sum_n Z[n, c] * A^T[n, m] on the tensor engine
    (lhsT = Z tile stationary, rhs = A^T tile moving, fp32 PSUM accum).
  * The inter-layer exchange (ncfw AllGather of each core's z1 slice)
    is the critical path: the first collective completes ~110 us into
    the kernel (ncfw first-op cost, absorbed by a tiny warmup fired at
    engine boot) and later ones land serially ~15-20 us apart.  The
    kernel shapes all deadlines around that:
      - layer 1 runs as THREE m-passes of 256/256/512 columns, each
        followed by weight-apply/relu/store and an AllGather of that
        slice (triggers ~45/61/94 us);
      - layer 2 runs as paired m-half sub-passes per gather phase with
        consumption 16.8/16.8/33.6 us -- the serial chain beats every
        deadline with slack, exposing only the first gather.
  * Ring discipline: z0 + z1_loc stores + outputs on distinct rings
    from the A load and the gather reloads; reloads park at the tail
    of the sync ring where their collective waits can block nothing.
  * Layer-2 ordering is enforced by a hard PSUM dependency, not
    priorities: its first-half accumulators reuse layer-1 pass-2 PSUM
    tags, so L2's first matmul (which may park on a reload wait in the
    in-order PE queue) can never be scheduled ahead of layer-1's final
    tail (the scheduler orders by modeled ready-time and models
    collectives as instant -- priorities alone are unreliable).
  * Both c-halves of the 256-wide passes share one PSUM bank
    (per-element has_written, single start=True per bank).
    PSUM: 1 (L1 p0/p1) + 2 (L1 p2 / L2 mh0) + 2 (L2 mh1) + 2 (apply).
  * bf16 operands / fp32 accumulation (~0.5% rel-l2 vs fp32 ref).
"""

import contextlib

import numpy as np
import ml_dtypes

import concourse.mybir as mybir
import concourse.tile as tile
from concourse import bacc
from concourse.bass_utils import run_bass_kernel_spmd

BF16 = ml_dtypes.bfloat16

NCORES = 8
B, N, F, L = 4, 8192, 64, 2
C = B * F                      # 256 columns of the Z matrix
M_CORE = N // NCORES           # 1024 output rows per core
NT = N // 128                  # 64 contraction tiles of 128
MT = M_CORE // 128             # 8 output-row tiles of 128 per core
ZCH = 8                        # DMA chunks for z0
TPZ = NT // ZCH                # 8 n-tiles per z chunk
PW = [256, 256, 512]           # layer-1 pass widths (m columns)
PK = [2, 2, 4]                 # A chunks (2 MB) per pass
# gather slices in m-tiles, aligned with the passes
GSLICE = [(0, 2), (2, 4), (4, 8)]

_CACHED = {}


def _build_program():
    nc = bacc.Bacc("TRN2", target_bir_lowering=False, debug=False,
                   num_devices=NCORES)
    dt = mybir.dt

    at_d = nc.dram_tensor("at", [sum(PK) * 128, 8192], dt.bfloat16,
                          kind="ExternalInput")
    z0_d = nc.dram_tensor("z0", [ZCH * 128, TPZ * C], dt.bfloat16,
                          kind="ExternalInput")
    w_d = nc.dram_tensor("w", [128, 2 * 128], dt.bfloat16, kind="ExternalInput")
    out_d = nc.dram_tensor("out", [M_CORE, C], dt.bfloat16, kind="ExternalOutput")

    z1_loc = nc.dram_tensor("z1_loc", [M_CORE, C], dt.bfloat16)
    warm_in = nc.dram_tensor("warm_in", [1, 128], dt.bfloat16)
    warm_out = nc.dram_tensor("warm_out", [NCORES, 128], dt.bfloat16,
                              addr_space="Shared")
    z1g = [nc.dram_tensor(f"z1g{g}", [NCORES * (hi - lo) * 128, C],
                          dt.bfloat16, addr_space="Shared")
           for g, (lo, hi) in enumerate(GSLICE)]

    with tile.TileContext(nc) as tc:
        with tc.tile_pool(name="a_res", bufs=1) as a_pool, \
             tc.tile_pool(name="z_res", bufs=1) as z_pool, \
             tc.tile_pool(name="z1_res", bufs=1) as z1_pool, \
             tc.tile_pool(name="wk", bufs=1) as w_pool, \
             tc.tile_pool(name="ps", bufs=1, space="PSUM") as ps_pool, \
             tc.tile_pool(name="pz", bufs=2, space="PSUM") as psz_pool, \
             tc.tile_pool(name="hsb", bufs=2) as hsb_pool, \
             tc.tile_pool(name="zout", bufs=4) as zout_pool:

            # Warm the ncfw collective path at engine boot (before any
            # load DMA): the first collective pays a huge one-time cost
            # which this absorbs under layer 1.  Input is never read.
            nc.gpsimd.collective_compute(
                "AllGather",
                mybir.AluOpType.bypass,
                replica_groups=[list(range(NCORES))],
                ins=[warm_in.ap().opt()],
                outs=[warm_out.ap().opt()],
            )

            w_sb = w_pool.tile([128, 2 * 128], dt.bfloat16, tag="w")
            nc.scalar.dma_start(out=w_sb[:], in_=w_d[:])

            # Resident A^T shard: all chunks are [128, 8192] bf16 (2 MB)
            # regardless of pass width (n-coverage varies instead).
            at_sb = [[a_pool.tile([128, 8192], dt.bfloat16,
                                  tag=f"at{p}_{k}", name=f"at_sb{p}_{k}")
                      for k in range(PK[p])] for p in range(3)]
            z_sb = [z_pool.tile([128, TPZ * C], dt.bfloat16,
                                tag=f"z{k}", name=f"z_sb{k}")
                    for k in range(ZCH)]
            z1_sb = [z1_pool.tile([128, NCORES * (hi - lo) * C], dt.bfloat16,
                                  tag=f"z1g{g}", name=f"z1_sb{g}")
                     for g, (lo, hi) in enumerate(GSLICE)]

            for k in range(ZCH):
                nc.scalar.dma_start(out=z_sb[k][:],
                                    in_=z0_d[k * 128:(k + 1) * 128, :])
            row = 0
            for p in range(3):
                for k in range(PK[p]):
                    nc.sync.dma_start(out=at_sb[p][k][:],
                                      in_=at_d[row:row + 128, :])
                    row += 128

            def z_tile(t, ch):
                """lhsT: Z[n-tile t, c-half ch] -> [128, 128] bf16."""
                k, tt = divmod(t, TPZ)
                return z_sb[k][:, tt * C + ch * 128: tt * C + ch * 128 + 128]

            def z2_tile(t, ch):
                """Same, from the gathered z1 slices."""
                cb, r = divmod(t, MT)
                g = next(i for i, (lo, hi) in enumerate(GSLICE) if lo <= r < hi)
                lo, hi = GSLICE[g]
                blk = cb * (hi - lo) + (r - lo)
                return z1_sb[g][:, blk * C + ch * 128: blk * C + ch * 128 + 128]

            def at_tile(t, p, off=0, width=None):
                """rhs: A^T[n-tile t, cols off:off+width of pass p]."""
                width = width or PW[p]
                tpc = NT // PK[p]
                k, tt = divmod(t, tpc)
                base = tt * PW[p] + off
                return at_sb[p][k][:, base:base + width]

            h_sb = [hsb_pool.tile([128, M_CORE], dt.bfloat16,
                                  tag=f"h{ch}", name=f"h_sb{ch}")
                    for ch in range(2)]
            # pass offsets into the m dimension (in units of 128-tiles)
            POFF = [0, 2, 4]

            def apply_store(li, lo, hi, on_slice_done):
                # weight apply + relu + store for m-tiles [lo, hi), in
                # 2-tile PSUM-bank groups; overlaps later matmuls.
                for g0 in range(lo, hi, 2):
                    z_ps = psz_pool.tile([128, 2 * C], dt.float32,
                                         tag="zps", name=f"z_ps_{li}_{g0}")
                    for j in range(2):
                        i = g0 + j
                        for ch in range(2):
                            nc.tensor.matmul(
                                z_ps[:, j * C + ch * 128:
                                     j * C + (ch + 1) * 128],
                                h_sb[ch][:, i * 128:(i + 1) * 128],
                                w_sb[:, li * 128:(li + 1) * 128],
                                start=(j == 0 and ch == 0), stop=True,
                            )
                    z_o = zout_pool.tile([128, 2 * C], dt.bfloat16,
                                         tag="zo", name=f"z_o_{li}_{g0}")
                    nc.scalar.activation(z_o[:], z_ps[:],
                                         mybir.ActivationFunctionType.Relu)
                    on_slice_done(g0, z_o)

            # ---- layer 1: three m-passes (256 / 256 / 512 cols) ----
            def l1_store(g0, z_o):
                nc.scalar.dma_start(
                    out=z1_loc.ap()[g0 * 128:(g0 + 2) * 128, :]
                        .rearrange("(t p) c -> p t c", p=128),
                    in_=z_o.rearrange("p (t c) -> p t c", c=C))

            def l1_gather(g):
                lo, hi = GSLICE[g]
                nc.gpsimd.collective_compute(
                    "AllGather",
                    mybir.AluOpType.bypass,
                    replica_groups=[list(range(NCORES))],
                    ins=[z1_loc.ap()[lo * 128:hi * 128, :].opt()],
                    outs=[z1g[g].ap().opt()],
                )
                # Reload parks at the sync-ring tail (behind the A
                # load): its collective wait can block nothing there.
                with tc.high_priority(offset=-1_000_000):
                    nc.sync.dma_start(
                        out=z1_sb[g].rearrange("p (cb t c) -> p cb t c",
                                               cb=NCORES, t=hi - lo),
                        in_=z1g[g].ap().rearrange("(cb t p) c -> p cb t c",
                                                  cb=NCORES, p=128))

            # passes 0/1 (256-wide): both c-halves pair-packed into one
            # shared PSUM bank (pass 1 WAR-waits pass 0's cast, ~0).
            l1_pair = [ps_pool.tile([128, 512], dt.float32, tag="hl1_01",
                                    name=f"l1_pair{p}") for p in range(2)]
            for p in range(2):
                for ti, t in enumerate(range(NT)):
                    for ch in range(2):
                        nc.tensor.matmul(
                            l1_pair[p][:, ch * 256:(ch + 1) * 256],
                            z_tile(t, ch),
                            at_tile(t, p),
                            start=(ti == 0 and ch == 0),
                            stop=(ti == NT - 1),
                        )
                with tc.high_priority():
                    for ch in range(2):
                        nc.vector.tensor_copy(
                            h_sb[ch][:, POFF[p] * 128:(POFF[p] + 2) * 128],
                            l1_pair[p][:, ch * 256:(ch + 1) * 256],
                        )
                    apply_store(0, POFF[p], POFF[p] + 2, l1_store)
                    l1_gather(p)

            # pass 2 (512-wide): one PSUM bank per c-half.
            l1_w = [ps_pool.tile([128, 512], dt.float32, tag=f"hl1c_{ch}",
                                 name=f"l1_w{ch}") for ch in range(2)]
            for ti, t in enumerate(range(NT)):
                for ch in range(2):
                    nc.tensor.matmul(
                        l1_w[ch][:],
                        z_tile(t, ch),
                        at_tile(t, 2),
                        start=(ti == 0),
                        stop=(ti == NT - 1),
                    )
            with tc.high_priority():
                for ch in range(2):
                    nc.vector.tensor_copy(h_sb[ch][:, 512:1024], l1_w[ch][:])
                apply_store(0, 4, 8, l1_store)
                l1_gather(2)

            # ---- layer 2: paired m-half sub-passes per gather phase ----
            tg = [[MT * cb + r for cb in range(NCORES) for r in range(lo, hi)]
                  for (lo, hi) in GSLICE]

            def l2_store(g0, z_o):
                nc.scalar.dma_start(
                    out=out_d.ap()[g0 * 128:(g0 + 2) * 128, :]
                        .rearrange("(t p) c -> p t c", p=128),
                    in_=z_o.rearrange("p (t c) -> p t c", c=C))

            # mh0 reuses layer-1 pass-2 PSUM tags: the bank WAR forces
            # L2's first matmul after layer-1's final cast.
            l2_ps = [[ps_pool.tile([128, 512], dt.float32,
                                   tag=(f"hl1c_{ch}" if mh == 0
                                        else f"hl2_{ch}"),
                                   name=f"l2_ps_{mh}{ch}")
                      for ch in range(2)] for mh in range(2)]
            # layer-2 m-half mh streams: list of (pass, col offset)
            MHREF = [[(0, 0), (1, 0)], [(2, 0), (2, 256)]]

            def l2_subpass(gp, mh):
                with tc.high_priority(offset=-1_000_000):
                    for ti, t in enumerate(tg[gp]):
                        for ch in range(2):
                            for si, (p, off) in enumerate(MHREF[mh]):
                                nc.tensor.matmul(
                                    l2_ps[mh][ch][:, si * 256:
                                                  (si + 1) * 256],
                                    z2_tile(t, ch),
                                    at_tile(t, p, off, 256),
                                    start=(gp == 0 and ti == 0 and si == 0),
                                    stop=(gp == 2 and ti == len(tg[2]) - 1),
                                )

            def l2_tail(mh):
                for ch in range(2):
                    nc.vector.tensor_copy(
                        h_sb[ch][:, mh * 512:(mh + 1) * 512],
                        l2_ps[mh][ch][:],
                    )
                apply_store(1, mh * 4, mh * 4 + 4, l2_store)

            l2_subpass(0, 0)
            l2_subpass(0, 1)
            l2_subpass(1, 0)
            l2_subpass(1, 1)
            l2_subpass(2, 0)
            l2_tail(0)
            l2_subpass(2, 1)
            l2_tail(1)

    nc.compile()
    return nc


def _prep_inputs(x, net_params, A):
    a_bf = A.astype(BF16)
    z0 = np.ascontiguousarray(x.transpose(1, 0, 2).reshape(N, C)).astype(BF16)
    # z0 in SBUF tile order [k, p, t, c] -> [ZCH*128, TPZ*C]
    z0_sw = np.ascontiguousarray(
        z0.reshape(ZCH, TPZ, 128, C).transpose(0, 2, 1, 3)
    ).reshape(ZCH * 128, TPZ * C)
    w = net_params.astype(np.float32).reshape(L, F, F).astype(BF16)
    # block-diagonal weight tile per layer: diag(W_l, W_l)
    w_sb = np.zeros((128, 2 * 128), dtype=BF16)
    for li in range(L):
        w_sb[0:F, li * 128:li * 128 + F] = w[li]
        w_sb[F:2 * F, li * 128 + F:li * 128 + 2 * F] = w[li]
    in_maps = []
    moff = [0, 256, 512]
    for j in range(NCORES):
        at_j = np.ascontiguousarray(a_bf[j * M_CORE:(j + 1) * M_CORE, :].T)
        # per-pass chunking in SBUF tile order [p][k][part, t, m]
        at_sw = np.zeros((sum(PK) * 128, 8192), dtype=BF16)
        row = 0
        for p in range(3):
            tpc = NT // PK[p]
            blk = at_j[:, moff[p]:moff[p] + PW[p]].reshape(
                PK[p], tpc, 128, PW[p])
            blk = blk.transpose(0, 2, 1, 3).reshape(PK[p] * 128, tpc * PW[p])
            at_sw[row:row + PK[p] * 128, :] = blk
            row += PK[p] * 128
        in_maps.append({"at": at_sw, "z0": z0_sw, "w": w_sb})
    return in_maps


def kernel(x, t, net_params, A):
    x = np.asarray(x)
    A = np.asarray(A)
    net_params = np.asarray(net_params)

    if "nc" not in _CACHED:
        _CACHED["nc"] = _build_program()
    nc = _CACHED["nc"]

    in_maps = _prep_inputs(x, net_params, A)
    _CACHED["in_maps"] = in_maps
    res = run_bass_kernel_spmd(nc, in_maps, list(range(NCORES)))
    full = np.concatenate([res.results[c]["out"] for c in range(NCORES)],
                          axis=0).astype(np.float32)
    return np.ascontiguousarray(full.reshape(N, B, F).transpose(1, 0, 2))
